# revision 19
# baseline (speedup 1.0000x reference)
"""EUNN cell (B=2048, H=1024, capacity=128) on 8 NeuronCores.

Strategy: the 128 Givens-rotation layers compose into a banded complex matrix
M = L_128...L_1 with bandwidth <= 128, i.e. block-tridiagonal in 128-blocks.
The tiny parameter preprocessing composes M on host (numpy, ~100 MFLOP);
the device kernel is the bandwidth-heavy part: out^T = (D_omega M) x^T as
fp16 TensorEngine matmuls with fp32 PSUM accumulation.

Sharding: 8 cores = 4 batch quarters x 2 hidden halves. Each core computes
out^T rows [4j*128, 4j*128+512) for batch columns [i*512, (i+1)*512):
48 matmuls of [K=128]x[N=512], 8 PSUM banks, ~90 instructions total.
"""
import numpy as np

H = 1024
B = 2048
CAP = 128
EH = H // 2
OH = (H - 1) // 2
EC = (CAP + 1) // 2
OC = CAP // 2
BAND = CAP
NC_CORES = 8
NB = H // 128          # 8 hidden blocks
NJ = 2                 # hidden halves
NI = 4                 # batch quarters
BCORE = B // NI        # 512 batch cols per core
RH = NB // NJ          # 4 r-blocks per core
CS = RH + 2            # 6 c-blocks per core slab (with halo + dummy pad)
NPAIR = RH * 3         # 12 (r, c) block pairs per core (some zero-padded)

_perm_even = np.arange(EH * 2).reshape(-1, 2)[:, ::-1].reshape(-1)
_perm_odd = np.concatenate(
    [[0], np.arange(1, OH * 2 + 1).reshape(-1, 2)[:, ::-1].reshape(-1), [OH * 2 + 1]]
)


def _interleave(a, b):
    return np.stack([a, b], axis=-1).reshape(-1)


def _layer_coeffs(even_theta, odd_theta, even_phi, odd_phi):
    ce, se = np.cos(even_theta), np.sin(even_theta)
    cpe, spe = np.cos(even_phi), np.sin(even_phi)
    co, so = np.cos(odd_theta), np.sin(odd_theta)
    cpo, spo = np.cos(odd_phi), np.sin(odd_phi)
    zE = np.zeros(EH)
    zO = np.zeros(OH)
    one = np.ones(1)
    zero = np.zeros(1)
    for t in range(EC):
        ect, est, ecp, esp = ce[t], se[t], cpe[t], spe[t]
        v1 = _interleave(esp * ect, ect) + 1j * _interleave(ecp * ect, zE)
        v2 = _interleave(-esp * est, est) + 1j * _interleave(-ecp * est, zE)
        yield v1, v2, _perm_even
        oct_, ost, ocp, osp = co[t], so[t], cpo[t], spo[t]
        v1 = np.concatenate([one, _interleave(osp * oct_, oct_), one]) + 1j * np.concatenate(
            [zero, _interleave(ocp * oct_, zO), zero]
        )
        v2 = np.concatenate([zero, _interleave(-osp * ost, ost), zero]) + 1j * np.concatenate(
            [zero, _interleave(-ocp * ost, zO), zero]
        )
        yield v1, v2, _perm_odd


def _compose_banded(even_theta, odd_theta, even_phi, odd_phi):
    """M = L_128...L_1 as band array bnd[i, d], column j = i + d - BAND.

    Layer update: new[i, d] = v1[i]*bnd[i, d] + v2[i]*bnd[perm[i], d - s[i]],
    s[i] = perm[i] - i. Both layer types pair adjacent rows, so the update
    splits into two strided halves with fixed +-1 column shifts.
    """
    W = 2 * BAND + 1
    bnd = np.zeros((H, W), np.complex64)
    bnd[:, BAND] = 1.0
    new = np.zeros_like(bnd)
    for v1, v2, perm in _layer_coeffs(even_theta, odd_theta, even_phi, odd_phi):
        if perm is _perm_even:
            lo, hi = 0, H  # pairs (0,1),(2,3),...
        else:
            lo, hi = 1, H - 1  # pairs (1,2),(3,4),...; rows 0, H-1 fixed
            new[0] = v1[0] * bnd[0]
            new[H - 1] = v1[H - 1] * bnd[H - 1]
        a = bnd[lo:hi:2]      # upper row of each pair (s=+1)
        b = bnd[lo + 1:hi:2]  # lower row of each pair (s=-1)
        v1a = v1[lo:hi:2, None]
        v2a = v2[lo:hi:2, None]
        v1b = v1[lo + 1:hi:2, None]
        v2b = v2[lo + 1:hi:2, None]
        na = new[lo:hi:2]
        nb = new[lo + 1:hi:2]
        # upper: partner is lower row, shifted right in d (d-1)
        np.multiply(v1a, a, out=na)
        na[:, 1:] += (v2a * b[:, :-1]).astype(np.complex64)
        # lower: partner is upper row, shifted left in d (d+1)
        np.multiply(v1b, b, out=nb)
        nb[:, :-1] += (v2b * a[:, 1:]).astype(np.complex64)
        bnd, new = new, bnd
    return bnd


def _banded_to_dense(bnd):
    M = np.zeros((H, H), bnd.dtype)
    rows = np.arange(H)
    for d in range(2 * BAND + 1):
        j = rows + d - BAND
        ok = (j >= 0) & (j < H)
        M[rows[ok], j[ok]] = bnd[ok, d]
    return M


_NC_CACHE = {}


def _build_device_kernel(reps=1):
    key = ("nc", reps)
    if key in _NC_CACHE:
        return _NC_CACHE[key]
    import concourse.tile as tile
    from concourse import bacc, mybir

    f16 = mybir.dt.float16
    f32 = mybir.dt.float32
    nc = bacc.Bacc("TRN2", target_bir_lowering=False, debug=False)
    # x^T slabs, re then im, each 6 hidden blocks (halo + pad) x 512 batch cols
    x_d = nc.dram_tensor("x", [2 * CS * 128, BCORE], f16, kind="ExternalInput").ap()
    # packed lhsT blocks: re pair p at [:, p*128:(p+1)*128], then im pairs
    m_d = nc.dram_tensor("m", [128, 2 * NPAIR * 128], f16, kind="ExternalInput").ap()
    # out^T slabs, re then im, each 4 r-blocks x 512 batch cols (f16: values are
    # fp32-accumulated in PSUM, final rounding ~2.4e-4 relative)
    y_d = nc.dram_tensor("y", [2 * RH * 128, BCORE], f16, kind="ExternalOutput").ap()

    x_v = x_d.rearrange("(q p) b -> p q b", p=128)  # q = 2*CS blocks

    with tile.TileContext(nc) as tc:
        with (
            tc.tile_pool(name="mp", bufs=1) as mpool,
            tc.tile_pool(name="xp", bufs=2 if reps > 1 else 1) as xpool,
            tc.tile_pool(name="op", bufs=2 if reps > 1 else 1) as opool,
            tc.tile_pool(name="pp", bufs=1, space="PSUM") as pspool,
        ):
            m_t = mpool.tile([128, 2 * NPAIR * 128], f16, tag="m")
            nc.sync.dma_start(m_t[:], m_d)

            def msl(p, im):
                off = (im * NPAIR + p) * 128
                return m_t[:, off : off + 128]

            for _rep in range(reps):
                x_t = xpool.tile([128, 2 * CS * BCORE], f16, tag="x")
                xr3 = x_t[:].rearrange("p (q b) -> p q b", q=2 * CS)
                half = CS // 2  # 3-block chunks; order: xre lo, xim lo, xre hi, xim hi
                for s in (0, 2, 1, 3):
                    q0 = (s % 2) * CS + (s // 2) * half
                    nc.sync.dma_start(xr3[:, q0 : q0 + half], x_v[:, q0 : q0 + half])
                ximn_t = xpool.tile([128, CS * BCORE], f16, tag="ximn")
                for s in range(2):
                    sl = slice(s * half * BCORE, (s + 1) * half * BCORE)
                    nc.vector.tensor_scalar_mul(
                        ximn_t[:, sl], x_t[:, CS * BCORE :][:, sl], -1.0
                    )

                def xre(cl):
                    return x_t[:, cl * BCORE : (cl + 1) * BCORE]

                def xim(cl):
                    return x_t[:, (CS + cl) * BCORE : (CS + cl + 1) * BCORE]

                def ximn(cl):
                    return ximn_t[:, cl * BCORE : (cl + 1) * BCORE]

                o_t = opool.tile([128, 2 * RH * BCORE], f16, tag="o")

                for rl in range(RH):
                    psr = pspool.tile([128, BCORE], f32, tag=f"psr{rl}")
                    psi = pspool.tile([128, BCORE], f32, tag=f"psi{rl}")
                    for k in range(3):
                        cl = rl + k  # slab col block (slab offset = r0 - 1)
                        p = rl * 3 + k
                        first = k == 0
                        last = k == 2
                        nc.tensor.matmul(psr[:], lhsT=msl(p, 0), rhs=xre(cl), start=first, stop=False)
                        nc.tensor.matmul(psi[:], lhsT=msl(p, 0), rhs=xim(cl), start=first, stop=False)
                        nc.tensor.matmul(psi[:], lhsT=msl(p, 1), rhs=xre(cl), start=False, stop=last)
                        nc.tensor.matmul(psr[:], lhsT=msl(p, 1), rhs=ximn(cl), start=False, stop=last)
                    osl_r = slice(rl * BCORE, (rl + 1) * BCORE)
                    osl_i = slice((RH + rl) * BCORE, (RH + rl + 1) * BCORE)
                    # split PSUM->SBUF copies across ScalarE and VectorE
                    if rl % 2 == 0:
                        nc.scalar.copy(o_t[:, osl_r], psr[:])
                        nc.vector.tensor_copy(o_t[:, osl_i], psi[:])
                    else:
                        nc.vector.tensor_copy(o_t[:, osl_r], psr[:])
                        nc.scalar.copy(o_t[:, osl_i], psi[:])
                y_v = y_d.rearrange("(q p) b -> p q b", p=128)
                o_r = o_t[:].rearrange("p (q b) -> p q b", q=2 * RH)
                for s in range(4):
                    nc.sync.dma_start(
                        y_v[:, s * 2 : s * 2 + 2], o_r[:, s * 2 : s * 2 + 2]
                    )
    nc.compile()
    _NC_CACHE[key] = nc
    return nc


def _host_prepare(x_re, x_im, omega, even_theta, odd_theta, even_phi, odd_phi):
    """Compose M, fold omega, build per-core packed inputs."""
    bnd = _compose_banded(
        even_theta.astype(np.float64),
        odd_theta.astype(np.float64),
        even_phi.astype(np.float64),
        odd_phi.astype(np.float64),
    )
    M = _banded_to_dense(bnd)
    w = omega.astype(np.float64)
    Mw = (np.cos(w) + 1j * np.sin(w))[:, None] * M
    Mre = Mw.real.astype(np.float32)
    Mim = Mw.imag.astype(np.float32)

    xreT = np.ascontiguousarray(x_re.T).astype(np.float16)  # [H, B]
    ximT = np.ascontiguousarray(x_im.T).astype(np.float16)

    in_maps = []
    for core in range(NC_CORES):
        j, i = divmod(core, NI)
        r0 = j * RH
        c0 = r0 - 1  # slab block offset (may be -1 / run past NB-1)
        bs = slice(i * BCORE, (i + 1) * BCORE)

        x_s = np.zeros((2 * CS * 128, BCORE), np.float16)
        lo = max(c0, 0)
        hi = min(c0 + CS, NB)
        x_s[(lo - c0) * 128 : (hi - c0) * 128] = xreT[lo * 128 : hi * 128, bs]
        x_s[(CS + lo - c0) * 128 : (CS + hi - c0) * 128] = ximT[
            lo * 128 : hi * 128, bs
        ]

        m_p = np.zeros((128, 2 * NPAIR * 128), np.float16)
        for rl in range(RH):
            r = r0 + rl
            for k in range(3):
                c = r - 1 + k
                if not (0 <= c < NB):
                    continue  # leave zero block
                p = rl * 3 + k
                blk_re = Mre[r * 128 : (r + 1) * 128, c * 128 : (c + 1) * 128]
                blk_im = Mim[r * 128 : (r + 1) * 128, c * 128 : (c + 1) * 128]
                m_p[:, p * 128 : (p + 1) * 128] = blk_re.T.astype(np.float16)
                m_p[:, (NPAIR + p) * 128 : (NPAIR + p + 1) * 128] = blk_im.T.astype(
                    np.float16
                )

        in_maps.append({"x": x_s, "m": m_p})
    return in_maps


def kernel(x_re, x_im, omega, even_theta, odd_theta, even_phi, odd_phi):
    from concourse.bass_utils import run_bass_kernel_spmd

    in_maps = _host_prepare(
        np.asarray(x_re, np.float32),
        np.asarray(x_im, np.float32),
        np.asarray(omega),
        np.asarray(even_theta),
        np.asarray(odd_theta),
        np.asarray(even_phi),
        np.asarray(odd_phi),
    )
    nc = _build_device_kernel()
    res = run_bass_kernel_spmd(nc, in_maps, core_ids=list(range(NC_CORES)))
    yreT = np.empty((H, B), np.float32)
    yimT = np.empty((H, B), np.float32)
    for core in range(NC_CORES):
        j, i = divmod(core, NI)
        rs = slice(j * RH * 128, (j + 1) * RH * 128)
        bs = slice(i * BCORE, (i + 1) * BCORE)
        y = res.results[core]["y"]
        yreT[rs, bs] = y[: RH * 128].astype(np.float32)
        yimT[rs, bs] = y[RH * 128 :].astype(np.float32)
    out_re = np.ascontiguousarray(yreT.T)
    out_im = np.ascontiguousarray(yimT.T)
    return out_re, out_im


# revision 20
# speedup vs baseline: 1.0442x; 1.0442x over previous
"""EUNN cell (B=2048, H=1024, capacity=128) on 8 NeuronCores.

Strategy: the 128 Givens-rotation layers compose into a banded complex matrix
M = L_128...L_1 with bandwidth <= 128, i.e. block-tridiagonal in 128-blocks.
The tiny parameter preprocessing composes M on host (numpy, ~100 MFLOP);
the device kernel is the bandwidth-heavy part: out^T = (D_omega M) x^T as
fp16 TensorEngine matmuls with fp32 PSUM accumulation.

Sharding: 8 cores = 4 batch quarters x 2 hidden halves. Each core computes
out^T rows [4j*128, 4j*128+512) for batch columns [i*512, (i+1)*512):
48 matmuls of [K=128]x[N=512], 8 PSUM banks, ~90 instructions total.
"""
import numpy as np

H = 1024
B = 2048
CAP = 128
EH = H // 2
OH = (H - 1) // 2
EC = (CAP + 1) // 2
OC = CAP // 2
BAND = CAP
NC_CORES = 8
NB = H // 128          # 8 hidden blocks
NJ = 2                 # hidden halves
NI = 4                 # batch quarters
BCORE = B // NI        # 512 batch cols per core
RH = NB // NJ          # 4 r-blocks per core
CS = RH + 2            # 6 c-blocks per core slab (with halo + dummy pad)
NPAIR = RH * 3         # 12 (r, c) block pairs per core (some zero-padded)

_perm_even = np.arange(EH * 2).reshape(-1, 2)[:, ::-1].reshape(-1)
_perm_odd = np.concatenate(
    [[0], np.arange(1, OH * 2 + 1).reshape(-1, 2)[:, ::-1].reshape(-1), [OH * 2 + 1]]
)


def _interleave(a, b):
    return np.stack([a, b], axis=-1).reshape(-1)


def _layer_coeffs(even_theta, odd_theta, even_phi, odd_phi):
    ce, se = np.cos(even_theta), np.sin(even_theta)
    cpe, spe = np.cos(even_phi), np.sin(even_phi)
    co, so = np.cos(odd_theta), np.sin(odd_theta)
    cpo, spo = np.cos(odd_phi), np.sin(odd_phi)
    zE = np.zeros(EH)
    zO = np.zeros(OH)
    one = np.ones(1)
    zero = np.zeros(1)
    for t in range(EC):
        ect, est, ecp, esp = ce[t], se[t], cpe[t], spe[t]
        v1 = _interleave(esp * ect, ect) + 1j * _interleave(ecp * ect, zE)
        v2 = _interleave(-esp * est, est) + 1j * _interleave(-ecp * est, zE)
        yield v1, v2, _perm_even
        oct_, ost, ocp, osp = co[t], so[t], cpo[t], spo[t]
        v1 = np.concatenate([one, _interleave(osp * oct_, oct_), one]) + 1j * np.concatenate(
            [zero, _interleave(ocp * oct_, zO), zero]
        )
        v2 = np.concatenate([zero, _interleave(-osp * ost, ost), zero]) + 1j * np.concatenate(
            [zero, _interleave(-ocp * ost, zO), zero]
        )
        yield v1, v2, _perm_odd


def _compose_banded(even_theta, odd_theta, even_phi, odd_phi):
    """M = L_128...L_1 as band array bnd[i, d], column j = i + d - BAND.

    Layer update: new[i, d] = v1[i]*bnd[i, d] + v2[i]*bnd[perm[i], d - s[i]],
    s[i] = perm[i] - i. Both layer types pair adjacent rows, so the update
    splits into two strided halves with fixed +-1 column shifts.
    """
    W = 2 * BAND + 1
    bnd = np.zeros((H, W), np.complex64)
    bnd[:, BAND] = 1.0
    new = np.zeros_like(bnd)
    for v1, v2, perm in _layer_coeffs(even_theta, odd_theta, even_phi, odd_phi):
        if perm is _perm_even:
            lo, hi = 0, H  # pairs (0,1),(2,3),...
        else:
            lo, hi = 1, H - 1  # pairs (1,2),(3,4),...; rows 0, H-1 fixed
            new[0] = v1[0] * bnd[0]
            new[H - 1] = v1[H - 1] * bnd[H - 1]
        a = bnd[lo:hi:2]      # upper row of each pair (s=+1)
        b = bnd[lo + 1:hi:2]  # lower row of each pair (s=-1)
        v1a = v1[lo:hi:2, None]
        v2a = v2[lo:hi:2, None]
        v1b = v1[lo + 1:hi:2, None]
        v2b = v2[lo + 1:hi:2, None]
        na = new[lo:hi:2]
        nb = new[lo + 1:hi:2]
        # upper: partner is lower row, shifted right in d (d-1)
        np.multiply(v1a, a, out=na)
        na[:, 1:] += (v2a * b[:, :-1]).astype(np.complex64)
        # lower: partner is upper row, shifted left in d (d+1)
        np.multiply(v1b, b, out=nb)
        nb[:, :-1] += (v2b * a[:, 1:]).astype(np.complex64)
        bnd, new = new, bnd
    return bnd


def _banded_to_dense(bnd):
    M = np.zeros((H, H), bnd.dtype)
    rows = np.arange(H)
    for d in range(2 * BAND + 1):
        j = rows + d - BAND
        ok = (j >= 0) & (j < H)
        M[rows[ok], j[ok]] = bnd[ok, d]
    return M


_NC_CACHE = {}


def _build_device_kernel(reps=1):
    key = ("nc", reps)
    if key in _NC_CACHE:
        return _NC_CACHE[key]
    import concourse.tile as tile
    from concourse import bacc, mybir

    f16 = mybir.dt.float16
    f32 = mybir.dt.float32
    nc = bacc.Bacc("TRN2", target_bir_lowering=False, debug=False)
    # x^T slabs, re then im, each 6 hidden blocks (halo + pad) x 512 batch cols
    x_d = nc.dram_tensor("x", [2 * CS * 128, BCORE], f16, kind="ExternalInput").ap()
    # packed lhsT blocks: re pair p at [:, p*128:(p+1)*128], then im pairs
    m_d = nc.dram_tensor("m", [128, 2 * NPAIR * 128], f16, kind="ExternalInput").ap()
    # out^T slabs, re then im, each 4 r-blocks x 512 batch cols (f16: values are
    # fp32-accumulated in PSUM, final rounding ~2.4e-4 relative)
    y_d = nc.dram_tensor("y", [2 * RH * 128, BCORE], f16, kind="ExternalOutput").ap()

    x_v = x_d.rearrange("(q p) b -> p q b", p=128)  # q = 2*CS blocks

    with tile.TileContext(nc) as tc:
        with (
            tc.tile_pool(name="mp", bufs=1) as mpool,
            tc.tile_pool(name="xp", bufs=2 if reps > 1 else 1) as xpool,
            tc.tile_pool(name="op", bufs=2 if reps > 1 else 1) as opool,
            tc.tile_pool(name="pp", bufs=1, space="PSUM") as pspool,
        ):
            m_t = mpool.tile([128, 2 * NPAIR * 128], f16, tag="m")

            def msl(p, im):
                off = (im * NPAIR + p) * 128
                return m_t[:, off : off + 128]

            for _rep in range(reps):
                x_t = xpool.tile([128, 2 * CS * BCORE], f16, tag="x")
                xr3 = x_t[:].rearrange("p (q b) -> p q b", q=2 * CS)
                half = CS // 2
                # interleave M chunks (re/im x lo/hi pair halves) with 3-block
                # x chunks (xre lo, xim lo, xre hi, xim hi) so the first matmul
                # group unblocks as early as possible
                for kind, s in (
                    ("m", 0), ("x", 0), ("m", 1), ("x", 2),
                    ("m", 2), ("x", 1), ("m", 3), ("x", 3),
                ):
                    if kind == "m":
                        if _rep == 0:
                            im, lohi = s % 2, s // 2
                            off = (im * NPAIR + lohi * 6) * 128
                            nc.sync.dma_start(
                                m_t[:, off : off + 6 * 128],
                                m_d[:, off : off + 6 * 128],
                            )
                    else:
                        q0 = (s % 2) * CS + (s // 2) * half
                        nc.sync.dma_start(
                            xr3[:, q0 : q0 + half], x_v[:, q0 : q0 + half]
                        )
                ximn_t = xpool.tile([128, CS * BCORE], f16, tag="ximn")
                for s in range(2):
                    sl = slice(s * half * BCORE, (s + 1) * half * BCORE)
                    nc.vector.tensor_scalar_mul(
                        ximn_t[:, sl], x_t[:, CS * BCORE :][:, sl], -1.0
                    )

                def xre(cl):
                    return x_t[:, cl * BCORE : (cl + 1) * BCORE]

                def xim(cl):
                    return x_t[:, (CS + cl) * BCORE : (CS + cl + 1) * BCORE]

                def ximn(cl):
                    return ximn_t[:, cl * BCORE : (cl + 1) * BCORE]

                o_t = opool.tile([128, 2 * RH * BCORE], f16, tag="o")

                for rl in range(RH):
                    psr = pspool.tile([128, BCORE], f32, tag=f"psr{rl}")
                    psi = pspool.tile([128, BCORE], f32, tag=f"psi{rl}")
                    for k in range(3):
                        cl = rl + k  # slab col block (slab offset = r0 - 1)
                        p = rl * 3 + k
                        first = k == 0
                        last = k == 2
                        nc.tensor.matmul(psr[:], lhsT=msl(p, 0), rhs=xre(cl), start=first, stop=False)
                        nc.tensor.matmul(psi[:], lhsT=msl(p, 0), rhs=xim(cl), start=first, stop=False)
                        nc.tensor.matmul(psi[:], lhsT=msl(p, 1), rhs=xre(cl), start=False, stop=last)
                        nc.tensor.matmul(psr[:], lhsT=msl(p, 1), rhs=ximn(cl), start=False, stop=last)
                    osl_r = slice(rl * BCORE, (rl + 1) * BCORE)
                    osl_i = slice((RH + rl) * BCORE, (RH + rl + 1) * BCORE)
                    # split PSUM->SBUF copies across ScalarE and VectorE
                    if rl % 2 == 0:
                        nc.scalar.copy(o_t[:, osl_r], psr[:])
                        nc.vector.tensor_copy(o_t[:, osl_i], psi[:])
                    else:
                        nc.vector.tensor_copy(o_t[:, osl_r], psr[:])
                        nc.scalar.copy(o_t[:, osl_i], psi[:])
                y_v = y_d.rearrange("(q p) b -> p q b", p=128)
                o_r = o_t[:].rearrange("p (q b) -> p q b", q=2 * RH)
                for s in range(4):
                    nc.sync.dma_start(
                        y_v[:, s * 2 : s * 2 + 2], o_r[:, s * 2 : s * 2 + 2]
                    )
    nc.compile()
    _NC_CACHE[key] = nc
    return nc


def _host_prepare(x_re, x_im, omega, even_theta, odd_theta, even_phi, odd_phi):
    """Compose M, fold omega, build per-core packed inputs."""
    bnd = _compose_banded(
        even_theta.astype(np.float64),
        odd_theta.astype(np.float64),
        even_phi.astype(np.float64),
        odd_phi.astype(np.float64),
    )
    M = _banded_to_dense(bnd)
    w = omega.astype(np.float64)
    Mw = (np.cos(w) + 1j * np.sin(w))[:, None] * M
    Mre = Mw.real.astype(np.float32)
    Mim = Mw.imag.astype(np.float32)

    xreT = np.ascontiguousarray(x_re.T).astype(np.float16)  # [H, B]
    ximT = np.ascontiguousarray(x_im.T).astype(np.float16)

    in_maps = []
    for core in range(NC_CORES):
        j, i = divmod(core, NI)
        r0 = j * RH
        c0 = r0 - 1  # slab block offset (may be -1 / run past NB-1)
        bs = slice(i * BCORE, (i + 1) * BCORE)

        x_s = np.zeros((2 * CS * 128, BCORE), np.float16)
        lo = max(c0, 0)
        hi = min(c0 + CS, NB)
        x_s[(lo - c0) * 128 : (hi - c0) * 128] = xreT[lo * 128 : hi * 128, bs]
        x_s[(CS + lo - c0) * 128 : (CS + hi - c0) * 128] = ximT[
            lo * 128 : hi * 128, bs
        ]

        m_p = np.zeros((128, 2 * NPAIR * 128), np.float16)
        for rl in range(RH):
            r = r0 + rl
            for k in range(3):
                c = r - 1 + k
                if not (0 <= c < NB):
                    continue  # leave zero block
                p = rl * 3 + k
                blk_re = Mre[r * 128 : (r + 1) * 128, c * 128 : (c + 1) * 128]
                blk_im = Mim[r * 128 : (r + 1) * 128, c * 128 : (c + 1) * 128]
                m_p[:, p * 128 : (p + 1) * 128] = blk_re.T.astype(np.float16)
                m_p[:, (NPAIR + p) * 128 : (NPAIR + p + 1) * 128] = blk_im.T.astype(
                    np.float16
                )

        in_maps.append({"x": x_s, "m": m_p})
    return in_maps


def kernel(x_re, x_im, omega, even_theta, odd_theta, even_phi, odd_phi):
    from concourse.bass_utils import run_bass_kernel_spmd

    in_maps = _host_prepare(
        np.asarray(x_re, np.float32),
        np.asarray(x_im, np.float32),
        np.asarray(omega),
        np.asarray(even_theta),
        np.asarray(odd_theta),
        np.asarray(even_phi),
        np.asarray(odd_phi),
    )
    nc = _build_device_kernel()
    res = run_bass_kernel_spmd(nc, in_maps, core_ids=list(range(NC_CORES)))
    yreT = np.empty((H, B), np.float32)
    yimT = np.empty((H, B), np.float32)
    for core in range(NC_CORES):
        j, i = divmod(core, NI)
        rs = slice(j * RH * 128, (j + 1) * RH * 128)
        bs = slice(i * BCORE, (i + 1) * BCORE)
        y = res.results[core]["y"]
        yreT[rs, bs] = y[: RH * 128].astype(np.float32)
        yimT[rs, bs] = y[RH * 128 :].astype(np.float32)
    out_re = np.ascontiguousarray(yreT.T)
    out_im = np.ascontiguousarray(yimT.T)
    return out_re, out_im
